# revision 15
# baseline (speedup 1.0000x reference)
"""ExtractSearchWindows Trainium2 kernel (8 NeuronCores, Bass/Tile).

out[b, h, w, dy*cv+dx, ky*8+kx] = uint8(P[b, h+off+dy+ky, w+off+dx+kx])
with P = zero-pad(inputs[:, 0], 7) and off = 3 - search_range.

Strategy: the output (196.6 MB u8) is a pure byte-replication of a tiny
input, so the kernel is HBM-write-bound (~69 us/core floor). Work is
sharded over (b, h): each of the 8 cores produces 48 output rows.

Host prep (tiny): pad+cast the 0.5 MB input to u8 and lay out, per core,
a 1.6 MB array of byte-shifted sub-rows "S" such that every device-side
expansion copy becomes a 4-byte-aligned strided uint32 tensor_copy
(phase-decomposed over w mod 4).  Device per core: 3 tiles x 128
segments (segment = 40-pixel row chunk); per tile DMA-in 540 KB, 2x
[20 strided u32 DVE copies -> 4 MB contiguous DMA-out] in final
(w, d, t) byte order.
"""
import numpy as np

K = 8
MAX_SR = 3
B, H, W = 2, 192, 320
TP = MAX_SR + K // 2          # 7 pad per side
PW = W + 2 * TP               # 334
NCORES = 8
ROWS_PER_CORE = (B * H) // NCORES   # 48
WSEG = 40
NWSEG = W // WSEG             # 8
NSEG = ROWS_PER_CORE * NWSEG  # 384
NTILE = NSEG // 128           # 3
NCH = 2                       # w-chunks per segment
WCH = WSEG // NCH             # 20
NA = WCH // 4                 # 5

_PROG_CACHE = {}


def _geom(sr):
    cv = 2 * sr + 1
    off = MAX_SR - sr
    nv = cv - 1 + K                  # source rows per output row
    nu = 4 + cv - 1                  # shifted sub-rows: phi + dx
    nj = 4 * (WSEG // 4 - 1) + (K - 1) + 1  # sub-row bytes (covers all chunks)
    nj = (nj + 3) // 4 * 4                  # pad to mult of 4 -> 44
    return cv, off, nv, nu, nj


def _make_s_host(x, sr):
    """x: (B,1,H,W) f32 -> per-core list of [NSEG, nv*nu*nj] u8 arrays."""
    cv, off, nv, nu, nj = _geom(sr)
    P = np.pad(x[:, 0], ((0, 0), (TP, TP), (TP, TP))).astype(np.uint8)
    cores = []
    for c in range(NCORES):
        b = (c * ROWS_PER_CORE) // H
        h0 = (c * ROWS_PER_CORE) % H
        flat = np.ascontiguousarray(P[b]).reshape(-1)
        base = (h0 + off) * PW + off
        s5 = np.lib.stride_tricks.as_strided(
            flat[base:], shape=(ROWS_PER_CORE, NWSEG, nv, nu, nj),
            strides=(PW, WSEG, PW, 1, 1))
        cores.append(np.ascontiguousarray(s5).reshape(NSEG, nv * nu * nj))
    return cores


def _build_program(sr):
    import concourse.bass as bass
    import concourse.bacc as bacc
    import concourse.mybir as mybir
    from concourse import tile

    cv, off, nv, nu, nj = _geom(sr)
    segb = nv * nu * nj
    segw = segb // 4
    out_seg_b = WSEG * cv * cv * K * K
    ch_b = out_seg_b // NCH
    ch_w = ch_b // 4
    d_i32 = cv * K * K // 4        # u32 per pixel per dy (= 80 for cv=5)
    pix_i32 = cv * cv * K * K // 4  # u32 per pixel (= 400 for cv=5)

    u8 = mybir.dt.uint8
    u32 = mybir.dt.uint32
    nc = bacc.Bacc("TRN2", debug=False)
    s_in = nc.declare_dram_parameter("s_in", [NSEG, segb], u8, isOutput=False)
    out = nc.declare_dram_parameter("out", [NSEG * out_seg_b], u8, isOutput=True)

    # NOTE: out-DMAs must keep exactly 128 descriptors (= partitions) per
    # dma_start: the SDMA engines only sustain ~27 GB/s each when a DMA
    # spans all 128 partitions (112 descs -> ~19 GB/s, measured).

    t_bufs = 3
    dma_outs = []
    # S lives outside the tile framework so its input DMAs can issue
    # BEFORE the TileContext entry barrier: the Sync engine is ready
    # ~2us after its preamble, so the 1.6 MB input streams while the
    # other engines are still in the startup barrier chain (~3us of
    # input latency removed from the critical path).  Manual semaphore
    # + explicit DVE waits replace the framework's dependency tracking;
    # the sem is cleared at program end so reruns of the NEFF are safe.
    s_sem = nc.alloc_semaphore("s_in_sem")
    S_t = nc.alloc_sbuf_tensor("S_all", [128, NTILE * segb], u8)
    S_ap = S_t.ap()
    nc.sync.dma_start(
        bass.AP(S_ap.tensor, 0, [[NTILE * segb, 128], [1, segb]]),
        bass.AP(s_in.ap().tensor, 0, [[segb, 128], [1, segb]])
    ).then_inc(s_sem, 16)
    # DVE blocks on tile-0 data BEFORE joining the TileContext entry
    # barrier (pre-tc waits are not simulated by the tile scheduler, so
    # no deadlock); the data lands while the barrier chain runs.
    nc.vector.wait_ge(s_sem, 16)
    with tile.TileContext(nc) as tc:
        with tc.tile_pool(name="tpool", bufs=t_bufs) as tp:
            # tiles 1..2 input INSIDE tc: the framework tracks this DMA
            # against the t>=1 copies' reads of S and emits their waits.
            rest_src = bass.AP(s_in.ap().tensor, 128 * segb,
                               [[segb, 128], [128 * segb, NTILE - 1],
                                [1, segb]])
            rest_dst = bass.AP(S_ap.tensor, segb,
                               [[NTILE * segb, 128], [segb, NTILE - 1],
                                [1, segb]])
            nc.sync.dma_start(rest_dst, rest_src)
            s32 = S_ap.bitcast(u32)
            for t in range(NTILE):
                for ch in range(NCH):
                    T = tp.tile([128, ch_b], u8)
                    t32 = T[:].bitcast(u32)
                    # Pipeline-fill stage: split the very first chunk into
                    # three a-groups so the first DMA-out launches after
                    # ~1/3 of the chunk's copies; also split tile 0's
                    # second chunk so its first piece issues before the
                    # chunk-0 drain completes (kills a ~2us engine bubble).
                    if t == 0 and ch == 0:
                        groups = [(0, 2), (2, 2), (4, 1)]
                    elif t == 0 and ch == 1:
                        groups = [(0, 3), (3, 2)]
                    else:
                        groups = [(0, NA)]
                    for gi, (a0, an) in enumerate(groups):
                        for dy in range(cv):
                            for phi in range(4):
                                src = bass.AP(
                                    s32.tensor,
                                    t * segw + dy * (nu * nj // 4)
                                    + phi * (nj // 4) + NA * ch + a0,
                                    [[NTILE * segw, 128],
                                     [nu * nj // 4, K],  # ky: next src row
                                     [1, an],            # a: +4 bytes
                                     [nj // 4, cv],      # dx: next sub-row
                                     [1, 2]])            # kx pair
                                dst = bass.AP(
                                    t32.tensor,
                                    4 * a0 * pix_i32 + phi * pix_i32
                                    + dy * d_i32,
                                    [[ch_w, 128],
                                     [2, K],             # ky: +8 bytes
                                     [4 * pix_i32, an],  # a: +4 pixels
                                     [K * K // 4, cv],   # dx: +64 bytes
                                     [1, 2]])            # kx pair
                                nc.vector.tensor_copy(dst, src)
                        gb0 = 4 * a0 * cv * cv * K * K   # group byte start
                        gbn = 4 * an * cv * cv * K * K   # group byte len
                        dst_hbm = bass.AP(
                            out.ap().tensor,
                            (t * 128) * out_seg_b + ch * ch_b + gb0,
                            [[out_seg_b, 128], [1, gbn]])
                        dma_outs.append(
                            nc.sync.dma_start(dst_hbm, T[0:128, gb0:gb0 + gbn]))
    nc.gpsimd.sem_clear(s_sem)
    nc.compile()
    return nc


def _numpy_fallback(x, sr):
    cv, off, _, _, _ = _geom(sr)
    P = np.pad(x[:, 0], ((0, 0), (TP, TP), (TP, TP))).astype(np.uint8)
    out = np.empty((B, H, W, cv * cv, K * K), np.uint8)
    for dy in range(cv):
        for dx in range(cv):
            for ky in range(K):
                for kx in range(K):
                    out[:, :, :, dy * cv + dx, ky * K + kx] = \
                        P[:, off + dy + ky:off + dy + ky + H,
                          off + dx + kx:off + dx + kx + W]
    return out


def kernel(inputs, search_range):
    from concourse.bass_utils import run_bass_kernel_spmd

    x = np.asarray(inputs, dtype=np.float32)
    sr = int(np.asarray(search_range))
    if sr != 2 or x.shape != (B, 1, H, W):
        return _numpy_fallback(x, sr)

    cv = 2 * sr + 1
    if sr not in _PROG_CACHE:
        _PROG_CACHE[sr] = _build_program(sr)
    nc = _PROG_CACHE[sr]

    s_cores = _make_s_host(x, sr)
    res = run_bass_kernel_spmd(
        nc, [{"s_in": s} for s in s_cores], list(range(NCORES)))
    outs = [np.asarray(res.results[c]["out"]) for c in range(NCORES)]
    return np.concatenate(outs).reshape(B, H, W, cv * cv, K * K)

